# revision 40
# baseline (speedup 1.0000x reference)
"""Trainium2 Bass kernel for nn_Attention_90967407330064.

Dense single-head spatial attention over x:[B,C,H,W] with 1x1-conv QKV:
  q = Wq@x+bq [B,64,N], k = Wk@x+bk, v = Wv@x+bv [B,256,N], N=H*W=4096
  out = v @ softmax(qT k / sqrt(N)) + x

Sharding: data-parallel over batch B=16 across 8 cores (2 batches/core).

Design (vs the ~393us v1 baseline; measured ~358us, rel err ~1e-3):
  - exp of the NxN scores is split across TWO engines, whole j-tile-pairs
    strictly alternating: even pairs get true exp on ScalarE; odd pairs
    get a Schraudolph-style bit trick on the DVE (bits = RNE(A*s + B)
    written as uint8, bit-viewed as fp8e4m3), which lands within the fp8
    mantissa quantization noise. This removes v1's ScalarE ACTIVATE wall
    (1308ns/block). Strict alternation matters: two consecutive pairs on
    one engine stall the 2-deep score-PSUM ring and (worse) the PE
    micro-idles re-throttle the HAM clock gate from 2.4 to 1.2 GHz --
    v1 ran at 1.2GHz throughout (462ns/512-col MM); this version holds
    2.4GHz (379ns) through the steady state.
  - all projections (q,k,v) run as single fp8 DoubleRow matmuls (K=256
    per instruction) off an fp8 copy of x. wqT/wkT stationaries have
    their 64 columns duplicated to M=128, so q and k come out of the
    projection already replicated into both partition halves for the
    row-packed score matmuls (matmul cost is column count -- free).
  - bv is folded into x in place once per batch (per-partition DVE add),
    so the tail is just po*(1/D) on DVE + a plain (+x') add on GpSimd.
  - PE work is emitted in runs (2 score-pairs, then the DR matmuls of
    the pair-group TRAIL groups back) to minimize rg<->128x128 mode
    switches and exposed LDWEIGHTS: sustained 216ns per 512-col DR MM
    (roofline) vs 233+ interleaved.
  - batch 1's x-DMA/casts/projections/vT production are injected into
    batch 0's main loop (prep slots per i-chunk), and batch 0's vT +
    projection chunks ride i-chunk 0's score groups, so the PE never
    sits behind a serial Scalar/DVE prep phase.
"""

import math
from contextlib import ExitStack

import numpy as np

import concourse.tile as tile
from concourse import bacc, mybir
from concourse.bass import ds, ts
from concourse.masks import make_identity

dt = mybir.dt

# Problem constants (hardcoded per harness contract).
B, C, H, W = 16, 256, 64, 64
DA = 64
N = H * W
N_CORES = 8
BPC = B // N_CORES  # batches per core

P = 128  # partitions
IC = 512  # i-chunk (psum bank width in fp32)

# Schraudolph fp8e4m3 exp constants: bits = RNE(A8*(s/sqrt(N)) + B8)
_SIGMA = 0.05
A8 = 8.0 / math.log(2.0)
B8 = 8.0 * (7.0 - _SIGMA)


def build_nc(bpc=BPC, c_dim=C, n_dim=N, da=DA, ic=IC):
    assert c_dim % P == 0 and n_dim % ic == 0 and n_dim % P == 0
    CT = c_dim // P  # c-tiles (2)
    KC = c_dim // P  # contraction chunks over c' (2)
    NIC = n_dim // ic  # i-chunks (8)
    NJT = n_dim // P  # j-tiles (32)
    NP = NJT // 2  # j-tile pairs (16)
    assert NP % 2 == 0
    inv_sqrt_n = 1.0 / math.sqrt(float(n_dim))
    aprime = A8 * inv_sqrt_n

    nc = bacc.Bacc(
        "TRN2", target_bir_lowering=False, debug=False, enable_asserts=False
    )
    f32, bf16, f8, u8 = dt.float32, dt.bfloat16, dt.float8e4, dt.uint8
    DR = mybir.MatmulPerfMode.DoubleRow

    x_d = nc.dram_tensor("x", [bpc, c_dim, n_dim], f32, kind="ExternalInput").ap()
    wq_d = nc.dram_tensor("Wq", [da, c_dim], f32, kind="ExternalInput").ap()
    bq_d = nc.dram_tensor("bq", [da], f32, kind="ExternalInput").ap()
    wk_d = nc.dram_tensor("Wk", [da, c_dim], f32, kind="ExternalInput").ap()
    bk_d = nc.dram_tensor("bk", [da], f32, kind="ExternalInput").ap()
    wv_d = nc.dram_tensor("Wv", [c_dim, c_dim], f32, kind="ExternalInput").ap()
    bv_d = nc.dram_tensor("bv", [c_dim], f32, kind="ExternalInput").ap()
    out_d = nc.dram_tensor("out", [bpc, c_dim, n_dim], f32, kind="ExternalOutput").ap()

    with tile.TileContext(nc) as tc, ExitStack() as ctx:
        consts = ctx.enter_context(tc.tile_pool(name="consts", bufs=1))
        xpool = ctx.enter_context(tc.tile_pool(name="xp", bufs=2))
        xf8p = ctx.enter_context(tc.tile_pool(name="xf8", bufs=2))
        qkp = ctx.enter_context(tc.tile_pool(name="qk", bufs=2))
        vTp = ctx.enter_context(tc.tile_pool(name="vT", bufs=2))
        et_pool = ctx.enter_context(tc.tile_pool(name="et", bufs=10))
        outs = ctx.enter_context(tc.tile_pool(name="outsb", bufs=3))
        small = ctx.enter_context(tc.tile_pool(name="small", bufs=2))
        # PSUM: ps_s 2 bufs x [P,1024] = 4 banks; po 3 bufs x 1 = 3; pd 1 -> 8
        ps_s = ctx.enter_context(tc.tile_pool(name="ps_s", bufs=2, space="PSUM"))
        ps_out = ctx.enter_context(tc.tile_pool(name="ps_out", bufs=3, space="PSUM"))
        ps_d = ctx.enter_context(tc.tile_pool(name="ps_d", bufs=1, space="PSUM"))

        # --- constants / weights (once per kernel) ---
        ident = consts.tile([P, P], f32)
        make_identity(nc, ident)
        ones_f8 = consts.tile([P, 2, P], f8, tag="ones")
        nc.vector.memset(ones_f8, 1.0)

        wq_sb = consts.tile([da, c_dim], f32, tag="wq")
        nc.sync.dma_start(wq_sb, wq_d)
        wk_sb = consts.tile([da, c_dim], f32, tag="wk")
        nc.sync.dma_start(wk_sb, wk_d)
        wv_sb = []
        for ct in range(CT):
            t = consts.tile([P, c_dim], f32, tag=f"wv{ct}")
            nc.sync.dma_start(t, wv_d[ts(ct, P), :])
            wv_sb.append(t)

        # bq/bk duplicated into both partition halves (q/k are produced
        # already-replicated by M=128 duplicated-column stationaries)
        bq_sb = consts.tile([P, 1], f32, tag="bq")
        nc.sync.dma_start(bq_sb[:da, :], bq_d.rearrange("(a o) -> a o", o=1))
        nc.sync.dma_start(bq_sb[da:, :], bq_d.rearrange("(a o) -> a o", o=1))
        bk_sb = consts.tile([P, 1], f32, tag="bk")
        nc.sync.dma_start(bk_sb[:da, :], bk_d.rearrange("(a o) -> a o", o=1))
        nc.sync.dma_start(bk_sb[da:, :], bk_d.rearrange("(a o) -> a o", o=1))
        # bv as [P, CT] for the tail scalar_tensor_tensor
        bv_sb = consts.tile([P, CT], f32, tag="bv")
        nc.sync.dma_start(bv_sb, bv_d.rearrange("(ct p) -> p ct", p=P))

        # PE warmup: burn the HAM cold window on the identity while the
        # first x DMA lands.
        warm_ps = ps_s.tile([P, 2 * ic], f32, tag="ps", name="warm_ps")
        for i in range(14):
            nc.tensor.matmul(
                warm_ps[:, ts(i % 2, P)], ident, ident, start=True, stop=True
            )

        # Transposed weights in fp8 DoubleRow layout. wqT/wkT have their
        # 64 columns DUPLICATED to M=128 so the projection matmul emits
        # q (k) already replicated into both partition halves for the
        # row-packed score matmuls -- free, matmul cost is column count.
        wqT_f8 = consts.tile([P, KC, P], f8, tag="wqT")
        wkT_f8 = consts.tile([P, KC, P], f8, tag="wkT")
        wvT_f8 = consts.tile([P, KC, c_dim], f8, tag="wvT")
        for kc in range(KC):
            pt = ps_s.tile([P, P], f32, tag="ps")
            nc.tensor.transpose(pt[:, :da], wq_sb[:, ts(kc, P)], ident[:da, :da])
            nc.scalar.copy(wqT_f8[:, kc, :da], pt[:, :da])
            nc.scalar.copy(wqT_f8[:, kc, da:], pt[:, :da])
            pt2 = ps_s.tile([P, P], f32, tag="ps")
            nc.tensor.transpose(pt2[:, :da], wk_sb[:, ts(kc, P)], ident[:da, :da])
            nc.scalar.copy(wkT_f8[:, kc, :da], pt2[:, :da])
            nc.scalar.copy(wkT_f8[:, kc, da:], pt2[:, :da])
            for ct in range(CT):
                pt3 = ps_s.tile([P, P], f32, tag="ps")
                nc.tensor.transpose(pt3, wv_sb[ct][:, ts(kc, P)], ident)
                nc.scalar.copy(wvT_f8[:, kc, ts(ct, P)], pt3)

        # ---------- per-batch emission pieces (software-pipelined) ----------
        def emit_x_dma(b):
            st = {}
            st["x_sb"] = [
                xpool.tile([P, n_dim], f32, tag=f"x{ct}", name=f"xt{ct}")
                for ct in range(CT)
            ]
            for half in range(4):
                for ct in range(CT):
                    nc.sync.dma_start(
                        st["x_sb"][ct][:, ts(half, n_dim // 4)],
                        x_d[b, ts(ct, P), ts(half, n_dim // 4)],
                    )
            return st

        def emit_x_cast_q(st, ct, qtr):
            if ct == 0 and qtr == 0:
                st["x_f8"] = xf8p.tile([P, KC, n_dim], f8, tag="xf8", name="xf8")
            nc.vector.tensor_copy(
                st["x_f8"][:, ct, ts(qtr, n_dim // 4)],
                st["x_sb"][ct][:, ts(qtr, n_dim // 4)],
            )

        def emit_x_cast(st, ct):
            for qtr in range(4):
                emit_x_cast_q(st, ct, qtr)

        def emit_bv_fold_h(st, ct, h):
            # fold bv into x in place (after the fp8 snapshot): the tail
            # then only needs a plain 2-tensor add (GpSimd-legal)
            nc.vector.tensor_scalar_add(
                st["x_sb"][ct][:, ts(h, n_dim // 4)],
                st["x_sb"][ct][:, ts(h, n_dim // 4)],
                bv_sb[:, ds(ct, 1)],
            )

        def emit_bv_fold(st, ct):
            for h in range(2):
                nc.vector.tensor_scalar_add(
                    st["x_sb"][ct][:, ts(h, n_dim // 2)],
                    st["x_sb"][ct][:, ts(h, n_dim // 2)],
                    bv_sb[:, ds(ct, 1)],
                )

        def emit_projk_chunk(st, n_i, dve_k=True):
            if n_i == 0:
                st["q_sb"] = qkp.tile([P, n_dim], f8, tag="q", name="q_sb")
                st["k_sb"] = qkp.tile([P, n_dim], f8, tag="k", name="k_sb")
            pk = ps_s.tile([P, ic], f32, tag="ps", name="pk")
            nc.tensor.matmul(
                pk, wkT_f8, st["x_f8"][:, :, ts(n_i, ic)],
                start=True, stop=True, perf_mode=DR,
            )
            if dve_k:
                nc.vector.tensor_scalar_add(st["k_sb"][:, ts(n_i, ic)], pk, bk_sb)
            else:
                nc.scalar.activation(
                    st["k_sb"][:, ts(n_i, ic)], pk,
                    mybir.ActivationFunctionType.Identity, bias=bk_sb,
                )

        def emit_projq_chunk(st, n_i):
            pq = ps_s.tile([P, ic], f32, tag="ps", name="pq")
            nc.tensor.matmul(
                pq, wqT_f8, st["x_f8"][:, :, ts(n_i, ic)],
                start=True, stop=True, perf_mode=DR,
            )
            nc.scalar.activation(
                st["q_sb"][:, ts(n_i, ic)], pq,
                mybir.ActivationFunctionType.Identity, bias=bq_sb,
            )

        def emit_proj_chunk(st, n_i, dve_k=True):
            emit_projk_chunk(st, n_i, dve_k=dve_k)
            emit_projq_chunk(st, n_i)

        def emit_vT_tile(st, t_j, dve=False):
            if t_j == 0:
                st["vT_sb"] = vTp.tile([P, NP, 2, c_dim], f8, tag="vT", name="vT_sb")
            pv = ps_s.tile([P, c_dim], f32, tag="ps", name="pv")
            nc.tensor.matmul(
                pv, st["x_f8"][:, :, ts(t_j, P)], wvT_f8,
                start=True, stop=True, perf_mode=DR,
            )
            if dve:
                nc.vector.tensor_copy(st["vT_sb"][:, t_j // 2, t_j % 2, :], pv)
            else:
                nc.scalar.copy(st["vT_sb"][:, t_j // 2, t_j % 2, :], pv)

        def emit_main_ic(b, st, i_c, prep=None, group_prep=None, dve_extra=False, trail=3):
            q_sb, k_sb, x_sb = st["q_sb"], st["k_sb"], st["x_sb"]
            po = [
                ps_out.tile([P, ic], f32, tag="o", name=f"po{c0}")
                for c0 in range(CT)
            ]
            pd = ps_d.tile([P, ic], f32, tag="d", name="pd")
            ets = [None] * NP

            def emit_scores(jp):
                ps_pair = ps_s.tile([P, 2 * ic], f32, tag="ps", name="ps_pair")
                nc.tensor.matmul(
                    ps_pair[:, ts(0, ic)],
                    k_sb[:da, ts(2 * jp, P)],
                    q_sb[:da, ts(i_c, ic)],
                    start=True, stop=True, tile_position=(0, 0),
                )
                nc.tensor.matmul(
                    ps_pair[:, ts(1, ic)],
                    k_sb[da:, ts(2 * jp + 1, P)],
                    q_sb[da:, ts(i_c, ic)],
                    start=True, stop=True, tile_position=(da, 0),
                )
                et = et_pool.tile([P, 2, ic], f8, tag="et", name="et")
                if jp % 2 == 1:
                    # whole pair via Schraudolph bits on DVE
                    nc.vector.tensor_scalar(
                        et.bitcast(u8).rearrange("p two n -> p (two n)"),
                        ps_pair,
                        float(aprime), float(B8),
                        mybir.AluOpType.mult, mybir.AluOpType.add,
                    )
                else:
                    # whole pair via true exp on ScalarE
                    nc.scalar.activation(
                        et.rearrange("p two n -> p (two n)"), ps_pair,
                        mybir.ActivationFunctionType.Exp, scale=inv_sqrt_n,
                    )
                ets[jp] = et

            def emit_outs(jp):
                for c0 in range(CT):
                    nc.tensor.matmul(
                        po[c0],
                        st["vT_sb"][:, jp, :, ts(c0, P)],
                        ets[jp],
                        start=(jp == 0), stop=(jp == NP - 1),
                        perf_mode=DR, skip_group_check=True,
                    )
                nc.tensor.matmul(
                    pd, ones_f8, ets[jp],
                    start=(jp == 0), stop=(jp == NP - 1),
                    perf_mode=DR, skip_group_check=True,
                )

            NG = NP // 2
            TRAIL = trail
            # next-batch prep thunks are spread across groups to avoid
            # bursting the Scalar/DVE queues at one point
            chunks = [[] for _ in range(NG)]
            if prep is not None:
                for idx, f in enumerate(prep):
                    chunks[idx * NG // len(prep)].append(f)
            for g in range(NG):
                emit_scores(2 * g)
                emit_scores(2 * g + 1)
                if group_prep is not None:
                    group_prep(g)
                for f in chunks[g]:
                    f()
                if g >= TRAIL:
                    emit_outs(2 * (g - TRAIL))
                    emit_outs(2 * (g - TRAIL) + 1)
            for jp in range(2 * (NG - TRAIL), NP):
                emit_outs(jp)

            # tail: out = po/D + bv + x  (GpSimd does the 2-tensor add;
            # bv was pre-folded into x)
            rd = small.tile([P, ic], f32, tag="rd", name="rd")
            nc.vector.reciprocal_approx_fast(rd, pd)
            for c0 in range(CT):
                ob = outs.tile([P, ic], f32, tag="ob", name="ob")
                nc.vector.tensor_mul(ob, po[c0], rd)
                ob2 = outs.tile([P, ic], f32, tag="ob2", name="ob2")
                nc.gpsimd.tensor_add(ob2, ob, x_sb[c0][:, ts(i_c, ic)])
                nc.sync.dma_start(out_d[b, ts(c0, P), ts(i_c, ic)], ob2)

        # ---------- pipelined schedule over batches ----------
        states = [None] * bpc
        states[0] = emit_x_dma(0)
        for ct in range(CT):
            emit_x_cast(states[0], ct)
            emit_bv_fold(states[0], ct)
        emit_projk_chunk(states[0], 0, dve_k=True)
        emit_projq_chunk(states[0], 0)
        # warmup burst #2: keep HAM at 8/8 across the proj-write wait; real
        # dependencies only on x_f8 (already landed), no downstream engines.
        warm2 = ps_s.tile([P, ic], f32, tag="ps", name="warm2")
        for i in range(12):
            nc.tensor.matmul(
                warm2, ones_f8, states[0]["x_f8"][:, :, ts(i % 4, ic)],
                start=True, stop=True, perf_mode=DR, skip_group_check=True,
            )
        def ic0_group_prep(st):
            # group g: vT tiles 4g..4g+3 (needed by outs at g+trail) and
            # k-chunk g+1 (k-cols for group g+1; Scalar to spare the DVE).
            # q-chunks 2..7 are deferred to the i-chunk before each is used.
            def run(g):
                for t_j in range(4 * g, 4 * g + 4):
                    emit_vT_tile(st, t_j, dve=(t_j % 2 == 1))
                if g + 1 < NIC:
                    emit_projk_chunk(st, g + 1, dve_k=False)
                if g == 0:
                    emit_projq_chunk(st, 1)
            return run

        for b in range(bpc):
            nxt = b + 1 if b + 1 < bpc else None
            # next-batch prep chunks injected mid-ic (at the last score
            # group, so the PE picks them up while outs trail)
            preps = {}
            if nxt is not None:
                sn = states  # close over
                def prep_ic0():
                    sn[nxt] = emit_x_dma(nxt)
                preps[0] = [prep_ic0]
                preps[1] = [
                    lambda: emit_x_cast(sn[nxt], 0),
                    lambda: emit_bv_fold(sn[nxt], 0),
                ]
                preps[2] = [
                    lambda: emit_x_cast(sn[nxt], 1),
                    lambda: emit_bv_fold(sn[nxt], 1),
                ]
                preps[3] = [
                    lambda n_i=n_i: emit_proj_chunk(sn[nxt], n_i, dve_k=False)
                    for n_i in range(0, 4)
                ]
                preps[4] = [
                    lambda n_i=n_i: emit_proj_chunk(sn[nxt], n_i, dve_k=False)
                    for n_i in range(4, NIC)
                ]
                preps[5] = [
                    lambda t_j=t_j: emit_vT_tile(sn[nxt], t_j)
                    for t_j in range(0, 11)
                ]
                preps[6] = [
                    lambda t_j=t_j: emit_vT_tile(sn[nxt], t_j)
                    for t_j in range(11, 22)
                ]
                preps[7] = [
                    lambda t_j=t_j: emit_vT_tile(sn[nxt], t_j)
                    for t_j in range(22, NJT)
                ]
            for i_c in range(NIC):
                if b == 0 and 1 <= i_c <= NIC - 2:
                    preps.setdefault(i_c, []).insert(
                        0, lambda n_i=i_c + 1: emit_projq_chunk(states[0], n_i)
                    )
                gp = None
                if b == 0 and i_c == 0:
                    gp = ic0_group_prep(states[0])
                emit_main_ic(
                    b, states[b], i_c,
                    prep=preps.get(i_c),
                    group_prep=gp,
                    dve_extra=(i_c >= 3 and b == 0 and nxt is not None),
                    trail=(2 if (nxt is None and i_c == NIC - 1) else 3),
                )

    nc.compile()
    return nc


_NC_CACHE = None


def get_nc():
    global _NC_CACHE
    if _NC_CACHE is None:
        _NC_CACHE = build_nc()
    return _NC_CACHE


def make_in_maps(inputs) -> list:
    x = np.ascontiguousarray(np.asarray(inputs["x"], dtype=np.float32)).reshape(
        B, C, N
    )
    w = {
        name: np.ascontiguousarray(np.asarray(inputs[name], dtype=np.float32))
        for name in ("Wq", "bq", "Wk", "bk", "Wv", "bv")
    }
    in_maps = []
    for c in range(N_CORES):
        m = {"x": np.ascontiguousarray(x[c * BPC : (c + 1) * BPC])}
        m.update(w)
        in_maps.append(m)
    return in_maps


def kernel(**inputs) -> np.ndarray:
    from concourse.bass_utils import run_bass_kernel_spmd

    res = run_bass_kernel_spmd(
        get_nc(), make_in_maps(inputs), core_ids=list(range(N_CORES))
    )
    out = np.concatenate([r["out"] for r in res.results], axis=0)
    return out.reshape(B, C, H, W).astype(np.float32)


# revision 41
# speedup vs baseline: 1.2002x; 1.2002x over previous
"""Trainium2 Bass kernel for nn_Attention_90967407330064.

Dense single-head spatial attention over x:[B,C,H,W] with 1x1-conv QKV:
  q = Wq@x+bq [B,64,N], k = Wk@x+bk, v = Wv@x+bv [B,256,N], N=H*W=4096
  out = v @ softmax(qT k / sqrt(N)) + x

Sharding: data-parallel over batch B=16 across 8 cores (2 batches/core).

Design (vs the ~393us v1 baseline; measured ~358us, rel err ~1e-3):
  - exp of the NxN scores is split across TWO engines, whole j-tile-pairs
    strictly alternating: even pairs get true exp on ScalarE; odd pairs
    get a Schraudolph-style bit trick on the DVE (bits = RNE(A*s + B)
    written as uint8, bit-viewed as fp8e4m3), which lands within the fp8
    mantissa quantization noise. This removes v1's ScalarE ACTIVATE wall
    (1308ns/block). Strict alternation matters: two consecutive pairs on
    one engine stall the 2-deep score-PSUM ring and (worse) the PE
    micro-idles re-throttle the HAM clock gate from 2.4 to 1.2 GHz --
    v1 ran at 1.2GHz throughout (462ns/512-col MM); this version holds
    2.4GHz (379ns) through the steady state.
  - all projections (q,k,v) run as single fp8 DoubleRow matmuls (K=256
    per instruction) off an fp8 copy of x. wqT/wkT stationaries have
    their 64 columns duplicated to M=128, so q and k come out of the
    projection already replicated into both partition halves for the
    row-packed score matmuls (matmul cost is column count -- free).
  - bv is folded into x in place once per batch (per-partition DVE add),
    so the tail is just po*(1/D) on DVE + a plain (+x') add on GpSimd.
  - PE work is emitted in runs (2 score-pairs, then the DR matmuls of
    the pair-group TRAIL groups back) to minimize rg<->128x128 mode
    switches and exposed LDWEIGHTS: sustained 216ns per 512-col DR MM
    (roofline) vs 233+ interleaved.
  - batch 1's x-DMA/casts/projections/vT production are injected into
    batch 0's main loop (prep slots per i-chunk), and batch 0's vT +
    projection chunks ride i-chunk 0's score groups, so the PE never
    sits behind a serial Scalar/DVE prep phase.
"""

import math
from contextlib import ExitStack

import numpy as np

import concourse.tile as tile
from concourse import bacc, mybir
from concourse.bass import ds, ts
from concourse.masks import make_identity

dt = mybir.dt

# Problem constants (hardcoded per harness contract).
B, C, H, W = 16, 256, 64, 64
DA = 64
N = H * W
N_CORES = 8
BPC = B // N_CORES  # batches per core

P = 128  # partitions
IC = 512  # i-chunk (psum bank width in fp32)

# Schraudolph fp8e4m3 exp constants: bits = RNE(A8*(s/sqrt(N)) + B8)
_SIGMA = 0.05
A8 = 8.0 / math.log(2.0)
B8 = 8.0 * (7.0 - _SIGMA)


def build_nc(bpc=BPC, c_dim=C, n_dim=N, da=DA, ic=IC):
    assert c_dim % P == 0 and n_dim % ic == 0 and n_dim % P == 0
    CT = c_dim // P  # c-tiles (2)
    KC = c_dim // P  # contraction chunks over c' (2)
    NIC = n_dim // ic  # i-chunks (8)
    NJT = n_dim // P  # j-tiles (32)
    NP = NJT // 2  # j-tile pairs (16)
    assert NP % 2 == 0
    inv_sqrt_n = 1.0 / math.sqrt(float(n_dim))
    aprime = A8 * inv_sqrt_n

    nc = bacc.Bacc(
        "TRN2", target_bir_lowering=False, debug=False, enable_asserts=False
    )
    f32, bf16, f8, u8 = dt.float32, dt.bfloat16, dt.float8e4, dt.uint8
    DR = mybir.MatmulPerfMode.DoubleRow

    x_d = nc.dram_tensor("x", [bpc, c_dim, n_dim], f32, kind="ExternalInput").ap()
    wq_d = nc.dram_tensor("Wq", [da, c_dim], f32, kind="ExternalInput").ap()
    bq_d = nc.dram_tensor("bq", [da], f32, kind="ExternalInput").ap()
    wk_d = nc.dram_tensor("Wk", [da, c_dim], f32, kind="ExternalInput").ap()
    bk_d = nc.dram_tensor("bk", [da], f32, kind="ExternalInput").ap()
    wv_d = nc.dram_tensor("Wv", [c_dim, c_dim], f32, kind="ExternalInput").ap()
    bv_d = nc.dram_tensor("bv", [c_dim], f32, kind="ExternalInput").ap()
    out_d = nc.dram_tensor("out", [bpc, c_dim, n_dim], f32, kind="ExternalOutput").ap()

    with tile.TileContext(nc) as tc, ExitStack() as ctx:
        consts = ctx.enter_context(tc.tile_pool(name="consts", bufs=1))
        xpool = ctx.enter_context(tc.tile_pool(name="xp", bufs=2))
        xf8p = ctx.enter_context(tc.tile_pool(name="xf8", bufs=2))
        qkp = ctx.enter_context(tc.tile_pool(name="qk", bufs=2))
        vTp = ctx.enter_context(tc.tile_pool(name="vT", bufs=2))
        et_pool = ctx.enter_context(tc.tile_pool(name="et", bufs=10))
        outs = ctx.enter_context(tc.tile_pool(name="outsb", bufs=3))
        small = ctx.enter_context(tc.tile_pool(name="small", bufs=2))
        # PSUM: ps_s 2 bufs x [P,1024] = 4 banks; po 3 bufs x 1 = 3; pd 1 -> 8
        ps_s = ctx.enter_context(tc.tile_pool(name="ps_s", bufs=2, space="PSUM"))
        ps_out = ctx.enter_context(tc.tile_pool(name="ps_out", bufs=3, space="PSUM"))
        ps_d = ctx.enter_context(tc.tile_pool(name="ps_d", bufs=1, space="PSUM"))

        # --- constants / weights (once per kernel) ---
        ident = consts.tile([P, P], f32)
        make_identity(nc, ident)
        ones_f8 = consts.tile([P, 2, P], f8, tag="ones")
        nc.vector.memset(ones_f8, 1.0)

        wq_sb = consts.tile([da, c_dim], f32, tag="wq")
        nc.sync.dma_start(wq_sb, wq_d)
        wk_sb = consts.tile([da, c_dim], f32, tag="wk")
        nc.sync.dma_start(wk_sb, wk_d)
        wv_sb = []
        for ct in range(CT):
            t = consts.tile([P, c_dim], f32, tag=f"wv{ct}")
            nc.sync.dma_start(t, wv_d[ts(ct, P), :])
            wv_sb.append(t)

        # bq/bk duplicated into both partition halves (q/k are produced
        # already-replicated by M=128 duplicated-column stationaries)
        bq_sb = consts.tile([P, 1], f32, tag="bq")
        nc.sync.dma_start(bq_sb[:da, :], bq_d.rearrange("(a o) -> a o", o=1))
        nc.sync.dma_start(bq_sb[da:, :], bq_d.rearrange("(a o) -> a o", o=1))
        bk_sb = consts.tile([P, 1], f32, tag="bk")
        nc.sync.dma_start(bk_sb[:da, :], bk_d.rearrange("(a o) -> a o", o=1))
        nc.sync.dma_start(bk_sb[da:, :], bk_d.rearrange("(a o) -> a o", o=1))
        # bv as [P, CT] for the tail scalar_tensor_tensor
        bv_sb = consts.tile([P, CT], f32, tag="bv")
        nc.sync.dma_start(bv_sb, bv_d.rearrange("(ct p) -> p ct", p=P))

        # PE warmup: the HAM clock gate un-throttles (1.2 -> 2.4 GHz) only
        # after ~3.4us of CONTINUOUS PE busy, and both states are absorbing
        # in the main loop (micro-gaps neither re-throttle nor un-throttle).
        # 14 back-to-back fp32 MMs (~3.1us) was a coin flip -- one cold run
        # executed the ENTIRE kernel at 1.2GHz (429us vs 358us). 24 MMs
        # (~5.4us, still under the x-DMA shadow) guarantees warm entry.
        warm_ps = ps_s.tile([P, 2 * ic], f32, tag="ps", name="warm_ps")
        for i in range(24):
            nc.tensor.matmul(
                warm_ps[:, ts(i % 2, P)], ident, ident, start=True, stop=True
            )

        # Transposed weights in fp8 DoubleRow layout. wqT/wkT have their
        # 64 columns DUPLICATED to M=128 so the projection matmul emits
        # q (k) already replicated into both partition halves for the
        # row-packed score matmuls -- free, matmul cost is column count.
        wqT_f8 = consts.tile([P, KC, P], f8, tag="wqT")
        wkT_f8 = consts.tile([P, KC, P], f8, tag="wkT")
        wvT_f8 = consts.tile([P, KC, c_dim], f8, tag="wvT")
        for kc in range(KC):
            pt = ps_s.tile([P, P], f32, tag="ps")
            nc.tensor.transpose(pt[:, :da], wq_sb[:, ts(kc, P)], ident[:da, :da])
            nc.scalar.copy(wqT_f8[:, kc, :da], pt[:, :da])
            nc.scalar.copy(wqT_f8[:, kc, da:], pt[:, :da])
            pt2 = ps_s.tile([P, P], f32, tag="ps")
            nc.tensor.transpose(pt2[:, :da], wk_sb[:, ts(kc, P)], ident[:da, :da])
            nc.scalar.copy(wkT_f8[:, kc, :da], pt2[:, :da])
            nc.scalar.copy(wkT_f8[:, kc, da:], pt2[:, :da])
            for ct in range(CT):
                pt3 = ps_s.tile([P, P], f32, tag="ps")
                nc.tensor.transpose(pt3, wv_sb[ct][:, ts(kc, P)], ident)
                nc.scalar.copy(wvT_f8[:, kc, ts(ct, P)], pt3)

        # ---------- per-batch emission pieces (software-pipelined) ----------
        def emit_x_dma(b):
            st = {}
            st["x_sb"] = [
                xpool.tile([P, n_dim], f32, tag=f"x{ct}", name=f"xt{ct}")
                for ct in range(CT)
            ]
            for half in range(4):
                for ct in range(CT):
                    nc.sync.dma_start(
                        st["x_sb"][ct][:, ts(half, n_dim // 4)],
                        x_d[b, ts(ct, P), ts(half, n_dim // 4)],
                    )
            return st

        def emit_x_cast_q(st, ct, qtr):
            if ct == 0 and qtr == 0:
                st["x_f8"] = xf8p.tile([P, KC, n_dim], f8, tag="xf8", name="xf8")
            nc.vector.tensor_copy(
                st["x_f8"][:, ct, ts(qtr, n_dim // 4)],
                st["x_sb"][ct][:, ts(qtr, n_dim // 4)],
            )

        def emit_x_cast(st, ct):
            for qtr in range(4):
                emit_x_cast_q(st, ct, qtr)

        def emit_bv_fold_h(st, ct, h):
            # fold bv into x in place (after the fp8 snapshot): the tail
            # then only needs a plain 2-tensor add (GpSimd-legal)
            nc.vector.tensor_scalar_add(
                st["x_sb"][ct][:, ts(h, n_dim // 4)],
                st["x_sb"][ct][:, ts(h, n_dim // 4)],
                bv_sb[:, ds(ct, 1)],
            )

        def emit_bv_fold(st, ct):
            for h in range(2):
                nc.vector.tensor_scalar_add(
                    st["x_sb"][ct][:, ts(h, n_dim // 2)],
                    st["x_sb"][ct][:, ts(h, n_dim // 2)],
                    bv_sb[:, ds(ct, 1)],
                )

        def emit_projk_chunk(st, n_i, dve_k=True):
            if n_i == 0:
                st["q_sb"] = qkp.tile([P, n_dim], f8, tag="q", name="q_sb")
                st["k_sb"] = qkp.tile([P, n_dim], f8, tag="k", name="k_sb")
            pk = ps_s.tile([P, ic], f32, tag="ps", name="pk")
            nc.tensor.matmul(
                pk, wkT_f8, st["x_f8"][:, :, ts(n_i, ic)],
                start=True, stop=True, perf_mode=DR,
            )
            if dve_k:
                nc.vector.tensor_scalar_add(st["k_sb"][:, ts(n_i, ic)], pk, bk_sb)
            else:
                nc.scalar.activation(
                    st["k_sb"][:, ts(n_i, ic)], pk,
                    mybir.ActivationFunctionType.Identity, bias=bk_sb,
                )

        def emit_projq_chunk(st, n_i):
            pq = ps_s.tile([P, ic], f32, tag="ps", name="pq")
            nc.tensor.matmul(
                pq, wqT_f8, st["x_f8"][:, :, ts(n_i, ic)],
                start=True, stop=True, perf_mode=DR,
            )
            nc.scalar.activation(
                st["q_sb"][:, ts(n_i, ic)], pq,
                mybir.ActivationFunctionType.Identity, bias=bq_sb,
            )

        def emit_proj_chunk(st, n_i, dve_k=True):
            emit_projk_chunk(st, n_i, dve_k=dve_k)
            emit_projq_chunk(st, n_i)

        def emit_vT_tile(st, t_j, dve=False):
            if t_j == 0:
                st["vT_sb"] = vTp.tile([P, NP, 2, c_dim], f8, tag="vT", name="vT_sb")
            pv = ps_s.tile([P, c_dim], f32, tag="ps", name="pv")
            nc.tensor.matmul(
                pv, st["x_f8"][:, :, ts(t_j, P)], wvT_f8,
                start=True, stop=True, perf_mode=DR,
            )
            if dve:
                nc.vector.tensor_copy(st["vT_sb"][:, t_j // 2, t_j % 2, :], pv)
            else:
                nc.scalar.copy(st["vT_sb"][:, t_j // 2, t_j % 2, :], pv)

        def emit_main_ic(b, st, i_c, prep=None, group_prep=None, dve_extra=False, trail=3):
            q_sb, k_sb, x_sb = st["q_sb"], st["k_sb"], st["x_sb"]
            po = [
                ps_out.tile([P, ic], f32, tag="o", name=f"po{c0}")
                for c0 in range(CT)
            ]
            pd = ps_d.tile([P, ic], f32, tag="d", name="pd")
            ets = [None] * NP

            def emit_scores(jp):
                ps_pair = ps_s.tile([P, 2 * ic], f32, tag="ps", name="ps_pair")
                nc.tensor.matmul(
                    ps_pair[:, ts(0, ic)],
                    k_sb[:da, ts(2 * jp, P)],
                    q_sb[:da, ts(i_c, ic)],
                    start=True, stop=True, tile_position=(0, 0),
                )
                nc.tensor.matmul(
                    ps_pair[:, ts(1, ic)],
                    k_sb[da:, ts(2 * jp + 1, P)],
                    q_sb[da:, ts(i_c, ic)],
                    start=True, stop=True, tile_position=(da, 0),
                )
                et = et_pool.tile([P, 2, ic], f8, tag="et", name="et")
                if jp % 2 == 1:
                    # whole pair via Schraudolph bits on DVE
                    nc.vector.tensor_scalar(
                        et.bitcast(u8).rearrange("p two n -> p (two n)"),
                        ps_pair,
                        float(aprime), float(B8),
                        mybir.AluOpType.mult, mybir.AluOpType.add,
                    )
                else:
                    # whole pair via true exp on ScalarE
                    nc.scalar.activation(
                        et.rearrange("p two n -> p (two n)"), ps_pair,
                        mybir.ActivationFunctionType.Exp, scale=inv_sqrt_n,
                    )
                ets[jp] = et

            def emit_outs(jp):
                for c0 in range(CT):
                    nc.tensor.matmul(
                        po[c0],
                        st["vT_sb"][:, jp, :, ts(c0, P)],
                        ets[jp],
                        start=(jp == 0), stop=(jp == NP - 1),
                        perf_mode=DR, skip_group_check=True,
                    )
                nc.tensor.matmul(
                    pd, ones_f8, ets[jp],
                    start=(jp == 0), stop=(jp == NP - 1),
                    perf_mode=DR, skip_group_check=True,
                )

            NG = NP // 2
            TRAIL = trail
            # next-batch prep thunks are spread across groups to avoid
            # bursting the Scalar/DVE queues at one point
            chunks = [[] for _ in range(NG)]
            if prep is not None:
                for idx, f in enumerate(prep):
                    chunks[idx * NG // len(prep)].append(f)
            for g in range(NG):
                emit_scores(2 * g)
                emit_scores(2 * g + 1)
                if group_prep is not None:
                    group_prep(g)
                for f in chunks[g]:
                    f()
                if g >= TRAIL:
                    emit_outs(2 * (g - TRAIL))
                    emit_outs(2 * (g - TRAIL) + 1)
            for jp in range(2 * (NG - TRAIL), NP):
                emit_outs(jp)

            # tail: out = po/D + bv + x  (GpSimd does the 2-tensor add;
            # bv was pre-folded into x)
            rd = small.tile([P, ic], f32, tag="rd", name="rd")
            nc.vector.reciprocal_approx_fast(rd, pd)
            for c0 in range(CT):
                ob = outs.tile([P, ic], f32, tag="ob", name="ob")
                nc.vector.tensor_mul(ob, po[c0], rd)
                ob2 = outs.tile([P, ic], f32, tag="ob2", name="ob2")
                nc.gpsimd.tensor_add(ob2, ob, x_sb[c0][:, ts(i_c, ic)])
                nc.sync.dma_start(out_d[b, ts(c0, P), ts(i_c, ic)], ob2)

        # ---------- pipelined schedule over batches ----------
        states = [None] * bpc
        states[0] = emit_x_dma(0)
        for ct in range(CT):
            emit_x_cast(states[0], ct)
            emit_bv_fold(states[0], ct)
        emit_projk_chunk(states[0], 0, dve_k=True)
        emit_projq_chunk(states[0], 0)
        # warmup burst #2: keep HAM at 8/8 across the proj-write wait; real
        # dependencies only on x_f8 (already landed), no downstream engines.
        warm2 = ps_s.tile([P, ic], f32, tag="ps", name="warm2")
        for i in range(12):
            nc.tensor.matmul(
                warm2, ones_f8, states[0]["x_f8"][:, :, ts(i % 4, ic)],
                start=True, stop=True, perf_mode=DR, skip_group_check=True,
            )
        def ic0_group_prep(st):
            # group g: vT tiles 4g..4g+3 (needed by outs at g+trail) and
            # k-chunk g+1 (k-cols for group g+1; Scalar to spare the DVE).
            # q-chunks 2..7 are deferred to the i-chunk before each is used.
            def run(g):
                for t_j in range(4 * g, 4 * g + 4):
                    emit_vT_tile(st, t_j, dve=(t_j % 2 == 1))
                if g + 1 < NIC:
                    emit_projk_chunk(st, g + 1, dve_k=False)
                if g == 0:
                    emit_projq_chunk(st, 1)
            return run

        for b in range(bpc):
            nxt = b + 1 if b + 1 < bpc else None
            # next-batch prep chunks injected mid-ic (at the last score
            # group, so the PE picks them up while outs trail)
            preps = {}
            if nxt is not None:
                sn = states  # close over
                def prep_ic0():
                    sn[nxt] = emit_x_dma(nxt)
                preps[0] = [prep_ic0]
                preps[1] = [
                    lambda: emit_x_cast(sn[nxt], 0),
                    lambda: emit_bv_fold(sn[nxt], 0),
                ]
                preps[2] = [
                    lambda: emit_x_cast(sn[nxt], 1),
                    lambda: emit_bv_fold(sn[nxt], 1),
                ]
                preps[3] = [
                    lambda n_i=n_i: emit_proj_chunk(sn[nxt], n_i, dve_k=False)
                    for n_i in range(0, 4)
                ]
                preps[4] = [
                    lambda n_i=n_i: emit_proj_chunk(sn[nxt], n_i, dve_k=False)
                    for n_i in range(4, NIC)
                ]
                preps[5] = [
                    lambda t_j=t_j: emit_vT_tile(sn[nxt], t_j)
                    for t_j in range(0, 11)
                ]
                preps[6] = [
                    lambda t_j=t_j: emit_vT_tile(sn[nxt], t_j)
                    for t_j in range(11, 22)
                ]
                preps[7] = [
                    lambda t_j=t_j: emit_vT_tile(sn[nxt], t_j)
                    for t_j in range(22, NJT)
                ]
            for i_c in range(NIC):
                if b == 0 and 1 <= i_c <= NIC - 2:
                    preps.setdefault(i_c, []).insert(
                        0, lambda n_i=i_c + 1: emit_projq_chunk(states[0], n_i)
                    )
                gp = None
                if b == 0 and i_c == 0:
                    gp = ic0_group_prep(states[0])
                emit_main_ic(
                    b, states[b], i_c,
                    prep=preps.get(i_c),
                    group_prep=gp,
                    dve_extra=(i_c >= 3 and b == 0 and nxt is not None),
                    trail=(2 if (nxt is None and i_c == NIC - 1) else 3),
                )

    nc.compile()
    return nc


_NC_CACHE = None


def get_nc():
    global _NC_CACHE
    if _NC_CACHE is None:
        _NC_CACHE = build_nc()
    return _NC_CACHE


def make_in_maps(inputs) -> list:
    x = np.ascontiguousarray(np.asarray(inputs["x"], dtype=np.float32)).reshape(
        B, C, N
    )
    w = {
        name: np.ascontiguousarray(np.asarray(inputs[name], dtype=np.float32))
        for name in ("Wq", "bq", "Wk", "bk", "Wv", "bv")
    }
    in_maps = []
    for c in range(N_CORES):
        m = {"x": np.ascontiguousarray(x[c * BPC : (c + 1) * BPC])}
        m.update(w)
        in_maps.append(m)
    return in_maps


def kernel(**inputs) -> np.ndarray:
    from concourse.bass_utils import run_bass_kernel_spmd

    res = run_bass_kernel_spmd(
        get_nc(), make_in_maps(inputs), core_ids=list(range(N_CORES))
    )
    out = np.concatenate([r["out"] for r in res.results], axis=0)
    return out.reshape(B, C, H, W).astype(np.float32)


# revision 42
# speedup vs baseline: 1.2026x; 1.0020x over previous
"""Trainium2 Bass kernel for nn_Attention_90967407330064.

Dense single-head spatial attention over x:[B,C,H,W] with 1x1-conv QKV:
  q = Wq@x+bq [B,64,N], k = Wk@x+bk, v = Wv@x+bv [B,256,N], N=H*W=4096
  out = v @ softmax(qT k / sqrt(N)) + x

Sharding: data-parallel over batch B=16 across 8 cores (2 batches/core).

Design (vs the ~393us v1 baseline; measured ~358us, rel err ~1e-3):
  - exp of the NxN scores is split across TWO engines, whole j-tile-pairs
    strictly alternating: even pairs get true exp on ScalarE; odd pairs
    get a Schraudolph-style bit trick on the DVE (bits = RNE(A*s + B)
    written as uint8, bit-viewed as fp8e4m3), which lands within the fp8
    mantissa quantization noise. This removes v1's ScalarE ACTIVATE wall
    (1308ns/block). Strict alternation matters: two consecutive pairs on
    one engine stall the 2-deep score-PSUM ring and (worse) the PE
    micro-idles re-throttle the HAM clock gate from 2.4 to 1.2 GHz --
    v1 ran at 1.2GHz throughout (462ns/512-col MM); this version holds
    2.4GHz (379ns) through the steady state.
  - all projections (q,k,v) run as single fp8 DoubleRow matmuls (K=256
    per instruction) off an fp8 copy of x. wqT/wkT stationaries have
    their 64 columns duplicated to M=128, so q and k come out of the
    projection already replicated into both partition halves for the
    row-packed score matmuls (matmul cost is column count -- free).
  - bv is folded into x in place once per batch (per-partition DVE add),
    so the tail is just po*(1/D) on DVE + a plain (+x') add on GpSimd.
  - PE work is emitted in runs (2 score-pairs, then the DR matmuls of
    the pair-group TRAIL groups back) to minimize rg<->128x128 mode
    switches and exposed LDWEIGHTS: sustained 216ns per 512-col DR MM
    (roofline) vs 233+ interleaved.
  - batch 1's x-DMA/casts/projections/vT production are injected into
    batch 0's main loop (prep slots per i-chunk), and batch 0's vT +
    projection chunks ride i-chunk 0's score groups, so the PE never
    sits behind a serial Scalar/DVE prep phase.
"""

import math
from contextlib import ExitStack

import numpy as np

import concourse.tile as tile
from concourse import bacc, mybir
from concourse.bass import ds, ts
from concourse.masks import make_identity

dt = mybir.dt

# Problem constants (hardcoded per harness contract).
B, C, H, W = 16, 256, 64, 64
DA = 64
N = H * W
N_CORES = 8
BPC = B // N_CORES  # batches per core

P = 128  # partitions
IC = 512  # i-chunk (psum bank width in fp32)

# Schraudolph fp8e4m3 exp constants: bits = RNE(A8*(s/sqrt(N)) + B8)
_SIGMA = 0.05
A8 = 8.0 / math.log(2.0)
B8 = 8.0 * (7.0 - _SIGMA)


def build_nc(bpc=BPC, c_dim=C, n_dim=N, da=DA, ic=IC):
    assert c_dim % P == 0 and n_dim % ic == 0 and n_dim % P == 0
    CT = c_dim // P  # c-tiles (2)
    KC = c_dim // P  # contraction chunks over c' (2)
    NIC = n_dim // ic  # i-chunks (8)
    NJT = n_dim // P  # j-tiles (32)
    NP = NJT // 2  # j-tile pairs (16)
    assert NP % 2 == 0
    inv_sqrt_n = 1.0 / math.sqrt(float(n_dim))
    aprime = A8 * inv_sqrt_n

    nc = bacc.Bacc(
        "TRN2", target_bir_lowering=False, debug=False, enable_asserts=False
    )
    f32, bf16, f8, u8 = dt.float32, dt.bfloat16, dt.float8e4, dt.uint8
    DR = mybir.MatmulPerfMode.DoubleRow

    x_d = nc.dram_tensor("x", [bpc, c_dim, n_dim], f32, kind="ExternalInput").ap()
    wq_d = nc.dram_tensor("Wq", [da, c_dim], f32, kind="ExternalInput").ap()
    bq_d = nc.dram_tensor("bq", [da], f32, kind="ExternalInput").ap()
    wk_d = nc.dram_tensor("Wk", [da, c_dim], f32, kind="ExternalInput").ap()
    bk_d = nc.dram_tensor("bk", [da], f32, kind="ExternalInput").ap()
    wv_d = nc.dram_tensor("Wv", [c_dim, c_dim], f32, kind="ExternalInput").ap()
    bv_d = nc.dram_tensor("bv", [c_dim], f32, kind="ExternalInput").ap()
    out_d = nc.dram_tensor("out", [bpc, c_dim, n_dim], f32, kind="ExternalOutput").ap()

    with tile.TileContext(nc) as tc, ExitStack() as ctx:
        consts = ctx.enter_context(tc.tile_pool(name="consts", bufs=1))
        xpool = ctx.enter_context(tc.tile_pool(name="xp", bufs=2))
        xf8p = ctx.enter_context(tc.tile_pool(name="xf8", bufs=2))
        qkp = ctx.enter_context(tc.tile_pool(name="qk", bufs=2))
        vTp = ctx.enter_context(tc.tile_pool(name="vT", bufs=2))
        et_pool = ctx.enter_context(tc.tile_pool(name="et", bufs=10))
        outs = ctx.enter_context(tc.tile_pool(name="outsb", bufs=3))
        small = ctx.enter_context(tc.tile_pool(name="small", bufs=2))
        # PSUM: ps_s 2 bufs x [P,1024] = 4 banks; po 3 bufs x 1 = 3; pd 1 -> 8
        ps_s = ctx.enter_context(tc.tile_pool(name="ps_s", bufs=2, space="PSUM"))
        ps_out = ctx.enter_context(tc.tile_pool(name="ps_out", bufs=3, space="PSUM"))
        ps_d = ctx.enter_context(tc.tile_pool(name="ps_d", bufs=1, space="PSUM"))

        # --- constants / weights (once per kernel) ---
        ident = consts.tile([P, P], f32)
        make_identity(nc, ident)
        ones_f8 = consts.tile([P, 2, P], f8, tag="ones")
        nc.vector.memset(ones_f8, 1.0)

        wq_sb = consts.tile([da, c_dim], f32, tag="wq")
        nc.sync.dma_start(wq_sb, wq_d)
        wk_sb = consts.tile([da, c_dim], f32, tag="wk")
        nc.sync.dma_start(wk_sb, wk_d)
        wv_sb = []
        for ct in range(CT):
            t = consts.tile([P, c_dim], f32, tag=f"wv{ct}")
            nc.sync.dma_start(t, wv_d[ts(ct, P), :])
            wv_sb.append(t)

        # bq/bk duplicated into both partition halves (q/k are produced
        # already-replicated by M=128 duplicated-column stationaries)
        bq_sb = consts.tile([P, 1], f32, tag="bq")
        nc.sync.dma_start(bq_sb[:da, :], bq_d.rearrange("(a o) -> a o", o=1))
        nc.sync.dma_start(bq_sb[da:, :], bq_d.rearrange("(a o) -> a o", o=1))
        bk_sb = consts.tile([P, 1], f32, tag="bk")
        nc.sync.dma_start(bk_sb[:da, :], bk_d.rearrange("(a o) -> a o", o=1))
        nc.sync.dma_start(bk_sb[da:, :], bk_d.rearrange("(a o) -> a o", o=1))
        # bv as [P, CT] for the tail scalar_tensor_tensor
        bv_sb = consts.tile([P, CT], f32, tag="bv")
        nc.sync.dma_start(bv_sb, bv_d.rearrange("(ct p) -> p ct", p=P))

        # PE warmup: the HAM clock gate un-throttles (1.2 -> 2.4 GHz) only
        # after ~3.4us of CONTINUOUS PE busy, and both states are absorbing
        # in the main loop (micro-gaps neither re-throttle nor un-throttle).
        # 14 back-to-back fp32 MMs (~3.1us) was a coin flip -- one cold run
        # executed the ENTIRE kernel at 1.2GHz (429us vs 358us). 24 MMs
        # (~5.4us, still under the x-DMA shadow) guarantees warm entry.
        warm_ps = ps_s.tile([P, 2 * ic], f32, tag="ps", name="warm_ps")
        for i in range(24):
            nc.tensor.matmul(
                warm_ps[:, ts(i % 2, P)], ident, ident, start=True, stop=True
            )

        # Transposed weights in fp8 DoubleRow layout. wqT/wkT have their
        # 64 columns DUPLICATED to M=128 so the projection matmul emits
        # q (k) already replicated into both partition halves for the
        # row-packed score matmuls -- free, matmul cost is column count.
        wqT_f8 = consts.tile([P, KC, P], f8, tag="wqT")
        wkT_f8 = consts.tile([P, KC, P], f8, tag="wkT")
        wvT_f8 = consts.tile([P, KC, c_dim], f8, tag="wvT")
        for kc in range(KC):
            pt = ps_s.tile([P, P], f32, tag="ps")
            nc.tensor.transpose(pt[:, :da], wq_sb[:, ts(kc, P)], ident[:da, :da])
            nc.scalar.copy(wqT_f8[:, kc, :da], pt[:, :da])
            nc.scalar.copy(wqT_f8[:, kc, da:], pt[:, :da])
            pt2 = ps_s.tile([P, P], f32, tag="ps")
            nc.tensor.transpose(pt2[:, :da], wk_sb[:, ts(kc, P)], ident[:da, :da])
            nc.scalar.copy(wkT_f8[:, kc, :da], pt2[:, :da])
            nc.scalar.copy(wkT_f8[:, kc, da:], pt2[:, :da])
            for ct in range(CT):
                pt3 = ps_s.tile([P, P], f32, tag="ps")
                nc.tensor.transpose(pt3, wv_sb[ct][:, ts(kc, P)], ident)
                nc.scalar.copy(wvT_f8[:, kc, ts(ct, P)], pt3)

        # ---------- per-batch emission pieces (software-pipelined) ----------
        def emit_x_dma(b):
            st = {}
            st["x_sb"] = [
                xpool.tile([P, n_dim], f32, tag=f"x{ct}", name=f"xt{ct}")
                for ct in range(CT)
            ]
            for half in range(4):
                for ct in range(CT):
                    nc.sync.dma_start(
                        st["x_sb"][ct][:, ts(half, n_dim // 4)],
                        x_d[b, ts(ct, P), ts(half, n_dim // 4)],
                    )
            return st

        def emit_x_cast_q(st, ct, qtr):
            if ct == 0 and qtr == 0:
                st["x_f8"] = xf8p.tile([P, KC, n_dim], f8, tag="xf8", name="xf8")
            nc.vector.tensor_copy(
                st["x_f8"][:, ct, ts(qtr, n_dim // 4)],
                st["x_sb"][ct][:, ts(qtr, n_dim // 4)],
            )

        def emit_x_cast(st, ct):
            for qtr in range(4):
                emit_x_cast_q(st, ct, qtr)

        def emit_bv_fold_h(st, ct, h):
            # fold bv into x in place (after the fp8 snapshot): the tail
            # then only needs a plain 2-tensor add (GpSimd-legal)
            nc.vector.tensor_scalar_add(
                st["x_sb"][ct][:, ts(h, n_dim // 4)],
                st["x_sb"][ct][:, ts(h, n_dim // 4)],
                bv_sb[:, ds(ct, 1)],
            )

        def emit_bv_fold(st, ct):
            for h in range(2):
                nc.vector.tensor_scalar_add(
                    st["x_sb"][ct][:, ts(h, n_dim // 2)],
                    st["x_sb"][ct][:, ts(h, n_dim // 2)],
                    bv_sb[:, ds(ct, 1)],
                )

        def emit_projk_chunk(st, n_i, dve_k=True):
            if n_i == 0:
                st["q_sb"] = qkp.tile([P, n_dim], f8, tag="q", name="q_sb")
                st["k_sb"] = qkp.tile([P, n_dim], f8, tag="k", name="k_sb")
            pk = ps_s.tile([P, ic], f32, tag="ps", name="pk")
            nc.tensor.matmul(
                pk, wkT_f8, st["x_f8"][:, :, ts(n_i, ic)],
                start=True, stop=True, perf_mode=DR,
            )
            if dve_k:
                nc.vector.tensor_scalar_add(st["k_sb"][:, ts(n_i, ic)], pk, bk_sb)
            else:
                nc.scalar.activation(
                    st["k_sb"][:, ts(n_i, ic)], pk,
                    mybir.ActivationFunctionType.Identity, bias=bk_sb,
                )

        def emit_projq_chunk(st, n_i):
            pq = ps_s.tile([P, ic], f32, tag="ps", name="pq")
            nc.tensor.matmul(
                pq, wqT_f8, st["x_f8"][:, :, ts(n_i, ic)],
                start=True, stop=True, perf_mode=DR,
            )
            nc.scalar.activation(
                st["q_sb"][:, ts(n_i, ic)], pq,
                mybir.ActivationFunctionType.Identity, bias=bq_sb,
            )

        def emit_proj_chunk(st, n_i, dve_k=True):
            emit_projk_chunk(st, n_i, dve_k=dve_k)
            emit_projq_chunk(st, n_i)

        def emit_vT_tile(st, t_j, dve=False):
            if t_j == 0:
                st["vT_sb"] = vTp.tile([P, NP, 2, c_dim], f8, tag="vT", name="vT_sb")
            pv = ps_s.tile([P, c_dim], f32, tag="ps", name="pv")
            nc.tensor.matmul(
                pv, st["x_f8"][:, :, ts(t_j, P)], wvT_f8,
                start=True, stop=True, perf_mode=DR,
            )
            if dve:
                nc.vector.tensor_copy(st["vT_sb"][:, t_j // 2, t_j % 2, :], pv)
            else:
                nc.scalar.copy(st["vT_sb"][:, t_j // 2, t_j % 2, :], pv)

        def emit_main_ic(b, st, i_c, prep=None, group_prep=None, dve_extra=False, trail=3, dve_tail=False):
            q_sb, k_sb, x_sb = st["q_sb"], st["k_sb"], st["x_sb"]
            po = [
                ps_out.tile([P, ic], f32, tag="o", name=f"po{c0}")
                for c0 in range(CT)
            ]
            pd = ps_d.tile([P, ic], f32, tag="d", name="pd")
            ets = [None] * NP

            def emit_scores(jp):
                ps_pair = ps_s.tile([P, 2 * ic], f32, tag="ps", name="ps_pair")
                nc.tensor.matmul(
                    ps_pair[:, ts(0, ic)],
                    k_sb[:da, ts(2 * jp, P)],
                    q_sb[:da, ts(i_c, ic)],
                    start=True, stop=True, tile_position=(0, 0),
                )
                nc.tensor.matmul(
                    ps_pair[:, ts(1, ic)],
                    k_sb[da:, ts(2 * jp + 1, P)],
                    q_sb[da:, ts(i_c, ic)],
                    start=True, stop=True, tile_position=(da, 0),
                )
                et = et_pool.tile([P, 2, ic], f8, tag="et", name="et")
                if jp % 2 == 1:
                    # whole pair via Schraudolph bits on DVE
                    nc.vector.tensor_scalar(
                        et.bitcast(u8).rearrange("p two n -> p (two n)"),
                        ps_pair,
                        float(aprime), float(B8),
                        mybir.AluOpType.mult, mybir.AluOpType.add,
                    )
                else:
                    # whole pair via true exp on ScalarE
                    nc.scalar.activation(
                        et.rearrange("p two n -> p (two n)"), ps_pair,
                        mybir.ActivationFunctionType.Exp, scale=inv_sqrt_n,
                    )
                ets[jp] = et

            def emit_outs(jp):
                for c0 in range(CT):
                    nc.tensor.matmul(
                        po[c0],
                        st["vT_sb"][:, jp, :, ts(c0, P)],
                        ets[jp],
                        start=(jp == 0), stop=(jp == NP - 1),
                        perf_mode=DR, skip_group_check=True,
                    )
                nc.tensor.matmul(
                    pd, ones_f8, ets[jp],
                    start=(jp == 0), stop=(jp == NP - 1),
                    perf_mode=DR, skip_group_check=True,
                )

            NG = NP // 2
            TRAIL = trail
            # next-batch prep thunks are spread across groups to avoid
            # bursting the Scalar/DVE queues at one point
            chunks = [[] for _ in range(NG)]
            if prep is not None:
                for idx, f in enumerate(prep):
                    chunks[idx * NG // len(prep)].append(f)
            for g in range(NG):
                emit_scores(2 * g)
                emit_scores(2 * g + 1)
                if group_prep is not None:
                    group_prep(g)
                for f in chunks[g]:
                    f()
                if g >= TRAIL:
                    emit_outs(2 * (g - TRAIL))
                    emit_outs(2 * (g - TRAIL) + 1)
            for jp in range(2 * (NG - TRAIL), NP):
                emit_outs(jp)

            # tail: out = po/D + bv + x  (GpSimd does the 2-tensor add;
            # bv was pre-folded into x)
            rd = small.tile([P, ic], f32, tag="rd", name="rd")
            nc.vector.reciprocal_approx_fast(rd, pd)
            for c0 in range(CT):
                ob = outs.tile([P, ic], f32, tag="ob", name="ob")
                nc.vector.tensor_mul(ob, po[c0], rd)
                ob2 = outs.tile([P, ic], f32, tag="ob2", name="ob2")
                if dve_tail:
                    # final i-chunk: DVE is idle and 2x faster than GpSimd
                    # here -- shortens the kernel's serial exit chain
                    nc.vector.tensor_add(ob2, ob, x_sb[c0][:, ts(i_c, ic)])
                else:
                    nc.gpsimd.tensor_add(ob2, ob, x_sb[c0][:, ts(i_c, ic)])
                nc.sync.dma_start(out_d[b, ts(c0, P), ts(i_c, ic)], ob2)

        # ---------- pipelined schedule over batches ----------
        states = [None] * bpc
        states[0] = emit_x_dma(0)
        for ct in range(CT):
            emit_x_cast(states[0], ct)
            emit_bv_fold(states[0], ct)
        emit_projk_chunk(states[0], 0, dve_k=True)
        emit_projq_chunk(states[0], 0)
        # warmup burst #2: keep HAM at 8/8 across the proj-write wait; real
        # dependencies only on x_f8 (already landed), no downstream engines.
        warm2 = ps_s.tile([P, ic], f32, tag="ps", name="warm2")
        for i in range(12):
            nc.tensor.matmul(
                warm2, ones_f8, states[0]["x_f8"][:, :, ts(i % 4, ic)],
                start=True, stop=True, perf_mode=DR, skip_group_check=True,
            )
        def ic0_group_prep(st):
            # group g: vT tiles 4g..4g+3 (needed by outs at g+trail) and
            # k-chunk g+1 (k-cols for group g+1; Scalar to spare the DVE).
            # q-chunks 2..7 are deferred to the i-chunk before each is used.
            def run(g):
                for t_j in range(4 * g, 4 * g + 4):
                    emit_vT_tile(st, t_j, dve=(t_j % 2 == 1))
                if g + 1 < NIC:
                    emit_projk_chunk(st, g + 1, dve_k=False)
                if g == 0:
                    emit_projq_chunk(st, 1)
            return run

        for b in range(bpc):
            nxt = b + 1 if b + 1 < bpc else None
            # next-batch prep chunks injected mid-ic (at the last score
            # group, so the PE picks them up while outs trail)
            preps = {}
            if nxt is not None:
                sn = states  # close over
                def prep_ic0():
                    sn[nxt] = emit_x_dma(nxt)
                preps[0] = [prep_ic0]
                preps[1] = [
                    lambda: emit_x_cast(sn[nxt], 0),
                    lambda: emit_bv_fold(sn[nxt], 0),
                ]
                preps[2] = [
                    lambda: emit_x_cast(sn[nxt], 1),
                    lambda: emit_bv_fold(sn[nxt], 1),
                ]
                preps[3] = [
                    lambda n_i=n_i: emit_proj_chunk(sn[nxt], n_i, dve_k=False)
                    for n_i in range(0, 4)
                ]
                preps[4] = [
                    lambda n_i=n_i: emit_proj_chunk(sn[nxt], n_i, dve_k=False)
                    for n_i in range(4, NIC)
                ]
                preps[5] = [
                    lambda t_j=t_j: emit_vT_tile(sn[nxt], t_j)
                    for t_j in range(0, 11)
                ]
                preps[6] = [
                    lambda t_j=t_j: emit_vT_tile(sn[nxt], t_j)
                    for t_j in range(11, 22)
                ]
                preps[7] = [
                    lambda t_j=t_j: emit_vT_tile(sn[nxt], t_j)
                    for t_j in range(22, NJT)
                ]
            for i_c in range(NIC):
                if b == 0 and 1 <= i_c <= NIC - 2:
                    preps.setdefault(i_c, []).insert(
                        0, lambda n_i=i_c + 1: emit_projq_chunk(states[0], n_i)
                    )
                gp = None
                if b == 0 and i_c == 0:
                    gp = ic0_group_prep(states[0])
                emit_main_ic(
                    b, states[b], i_c,
                    prep=preps.get(i_c),
                    group_prep=gp,
                    dve_extra=(i_c >= 3 and b == 0 and nxt is not None),
                    trail=(2 if (nxt is None and i_c == NIC - 1) else 3),
                    dve_tail=(nxt is None and i_c == NIC - 1),
                )

    nc.compile()
    return nc


_NC_CACHE = None


def get_nc():
    global _NC_CACHE
    if _NC_CACHE is None:
        _NC_CACHE = build_nc()
    return _NC_CACHE


def make_in_maps(inputs) -> list:
    x = np.ascontiguousarray(np.asarray(inputs["x"], dtype=np.float32)).reshape(
        B, C, N
    )
    w = {
        name: np.ascontiguousarray(np.asarray(inputs[name], dtype=np.float32))
        for name in ("Wq", "bq", "Wk", "bk", "Wv", "bv")
    }
    in_maps = []
    for c in range(N_CORES):
        m = {"x": np.ascontiguousarray(x[c * BPC : (c + 1) * BPC])}
        m.update(w)
        in_maps.append(m)
    return in_maps


def kernel(**inputs) -> np.ndarray:
    from concourse.bass_utils import run_bass_kernel_spmd

    res = run_bass_kernel_spmd(
        get_nc(), make_in_maps(inputs), core_ids=list(range(N_CORES))
    )
    out = np.concatenate([r["out"] for r in res.results], axis=0)
    return out.reshape(B, C, H, W).astype(np.float32)
